# revision 4
# baseline (speedup 1.0000x reference)
"""Trainium2 Bass kernel for nn_Classify_MLPPredictor (edge-parallel GNN inference).

Computes sigmoid(cat([h[src], h[dst]], -1) @ W + b) for E=1.6M edges over a
N=100k x 128 node table, on 8 NeuronCores.

Algorithm (per core, edges sharded 200k/core, h/W/b replicated, fp16 compute):
  Phase 1: pcat = h @ [Ws | Wd] + [0 | b]  -> one DRAM table [100k, 256] fp16
           (factored form: each node row is reused ~16x by the gather phase;
           fused [ps|pd] rows keep phase-1 store descriptors at 512B).
  Phase 2: per 128-edge tile, indirect-DMA gather of the ps half (src) and pd
           half (dst, element_offset=128) of pcat rows, fp16 add, sigmoid to
           fp32, store.  Gathers are spread round-robin over 4 SWDGE queues:
           the ~1us/instruction descriptor-generation cost is the bottleneck
           and parallelizes ~2.5x across queues (measured).
"""

import os
import time

import numpy as np

import concourse.bass as bass
import concourse.bacc as bacc
import concourse.mybir as mybir
import concourse.tile as tile
from concourse.bass_utils import run_bass_kernel_spmd

N_CORES = 8
N_NODES = 100000
D = 128           # feature dim
C = 128           # classes
CC = 2 * C        # concatenated output cols of phase 1
E = 1600000
E_C = E // N_CORES            # 200000 edges per core

# phase 1 tiling
P1_CHUNK = 1024               # nodes per DMA chunk (8 matmul subtiles)

# phase 2 tiling
TILE_E = 128                  # edges per gather
TILES_PER_BLK = 32            # gathers fused into one add/sigmoid/store block
N_TILES = 1563                # 1563*128 = 200064 >= 200000
E_PAD = N_TILES * TILE_E

NQ = 4                        # SWDGE queues for indirect gathers

F32 = mybir.dt.float32
F16 = mybir.dt.float16
I32 = mybir.dt.int32

_CACHE = {}


def _build_program(repeat=1):
    nc = bacc.Bacc(None, target_bir_lowering=False, num_swdge_queues=NQ)

    ht = nc.dram_tensor("ht", [D, N_NODES], F16, kind="ExternalInput")
    wcat = nc.dram_tensor("wcat", [D, CC], F16, kind="ExternalInput")
    bcat = nc.dram_tensor("bcat", [128, CC], F16, kind="ExternalInput")
    src_idx = nc.dram_tensor("src_idx", [128, N_TILES], I32, kind="ExternalInput")
    dst_idx = nc.dram_tensor("dst_idx", [128, N_TILES], I32, kind="ExternalInput")
    out = nc.dram_tensor("out", [E_PAD, C], F32, kind="ExternalOutput")

    pcat = nc.dram_tensor("pcat", [N_NODES, CC], F16, kind="Internal")

    with tile.TileContext(nc) as tc:
        with (
            tc.tile_pool(name="const", bufs=1) as cpool,
            tc.tile_pool(name="p1x", bufs=2) as xpool,
            tc.tile_pool(name="p1s", bufs=2) as spool,
            tc.tile_pool(name="psum", bufs=4, space="PSUM") as psum,
            tc.tile_pool(name="idx", bufs=1) as ipool,
            tc.tile_pool(name="g", bufs=2) as gpool,
            tc.tile_pool(name="o", bufs=2) as opool,
        ):
            wcat_t = cpool.tile([D, CC], F16)
            nc.sync.dma_start(out=wcat_t[:], in_=wcat[:])
            bcat_t = cpool.tile([128, CC], F16)
            nc.sync.dma_start(out=bcat_t[:], in_=bcat[:])

            # load all phase-2 indices up front (overlaps with phase 1)
            src_sb = ipool.tile([128, N_TILES], I32, tag="sidx")
            dst_sb = ipool.tile([128, N_TILES], I32, tag="didx")
            nc.sync.dma_start(out=src_sb[:], in_=src_idx[:])
            nc.sync.dma_start(out=dst_sb[:], in_=dst_idx[:])

            import contextlib

            rep_ctx = (
                tc.For_i(0, repeat, 1) if repeat > 1 else contextlib.nullcontext()
            )
            with rep_ctx:
                _emit_body(
                    nc, tc, xpool, spool, psum, gpool, opool,
                    ht, wcat_t, bcat_t, src_sb, dst_sb, pcat, out,
                )

    nc.compile()
    return nc


def _gather(nc, qctr, out_ap, table, idx_ap, element_offset):
    bi = nc.gpsimd.indirect_dma_start(
        out=out_ap,
        out_offset=None,
        in_=table[:, :],
        in_offset=bass.IndirectOffsetOnAxis(ap=idx_ap, axis=0),
        element_offset=element_offset,
    )
    q = qctr[0] % NQ
    qctr[0] += 1
    if q:
        bi.ins.queue = f"qPoolDynamic{q}"
    return bi


def _emit_body(nc, tc, xpool, spool, psum, gpool, opool,
               ht, wcat_t, bcat_t, src_sb, dst_sb, pcat, out):
    # ---------------- Phase 1: pcat = h @ [Ws|Wd] + [0|b] ----------------
    n0 = 0
    while n0 < N_NODES:
        nn = min(P1_CHUNK, N_NODES - n0)
        nsub = (nn + 127) // 128
        x = xpool.tile([D, P1_CHUNK], F16, tag="x")
        nc.sync.dma_start(out=x[:, :nn], in_=ht[:, n0 : n0 + nn])
        s = spool.tile([128, (P1_CHUNK // 128) * CC], F16, tag="s")
        for si in range(nsub):
            m = min(128, nn - si * 128)
            acc = psum.tile([128, CC], F32, tag="acc", space="PSUM")
            nc.tensor.matmul(
                acc[:m, :],
                lhsT=x[:, si * 128 : si * 128 + m],
                rhs=wcat_t[:],
                start=True,
                stop=True,
            )
            nc.vector.tensor_add(
                out=s[:m, si * CC : (si + 1) * CC],
                in0=acc[:m, :],
                in1=bcat_t[:m, :],
            )
        if nn == P1_CHUNK:
            sv = s[:].rearrange("p (s q) -> p s q", s=nsub)
            nc.sync.dma_start(
                out=pcat[n0 : n0 + nn, :].rearrange("(s p) c -> p s c", p=128),
                in_=sv[:],
            )
        else:
            for si in range(nsub):
                m = min(128, nn - si * 128)
                r0 = n0 + si * 128
                nc.sync.dma_start(
                    out=pcat[r0 : r0 + m, :],
                    in_=s[:m, si * CC : (si + 1) * CC],
                )
        n0 += nn

    # ---------------- Phase 2: gather + add + sigmoid + store -------------
    qctr = [0]
    t = 0
    while t < N_TILES:
        nt = min(TILES_PER_BLK, N_TILES - t)
        blk_w = nt * C
        gs = gpool.tile([128, TILES_PER_BLK * C], F16, tag="gs")
        gd = gpool.tile([128, TILES_PER_BLK * C], F16, tag="gd")
        for i in range(nt):
            tt = t + i
            _gather(nc, qctr, gs[:, i * C : (i + 1) * C], pcat,
                    src_sb[:, tt : tt + 1], 0)
            _gather(nc, qctr, gd[:, i * C : (i + 1) * C], pcat,
                    dst_sb[:, tt : tt + 1], C)
        o = opool.tile([128, TILES_PER_BLK * C], F32, tag="o")
        nc.vector.tensor_add(
            out=gs[:, :blk_w], in0=gs[:, :blk_w], in1=gd[:, :blk_w]
        )
        nc.scalar.activation(
            out=o[:, :blk_w],
            in_=gs[:, :blk_w],
            func=mybir.ActivationFunctionType.Sigmoid,
        )
        nc.sync.dma_start(
            out=out[t * TILE_E : (t + nt) * TILE_E, :].rearrange(
                "(i p) c -> p i c", p=128
            ),
            in_=o[:, :blk_w].rearrange("p (i c) -> p i c", c=C),
        )
        t += nt


def _prep_inputs(h, src, dst, W, b):
    h = np.asarray(h, dtype=np.float32)
    src = np.asarray(src)
    dst = np.asarray(dst)
    W = np.asarray(W, dtype=np.float32)
    b = np.asarray(b, dtype=np.float32)

    ht = np.ascontiguousarray(h.T.astype(np.float16))       # [128, 100000]
    wcat = np.ascontiguousarray(
        np.concatenate([W[:D], W[D:]], axis=1).astype(np.float16)  # [128, 256]
    )
    bcat = np.ascontiguousarray(
        np.tile(
            np.concatenate([np.zeros(C, np.float32), b])[None, :].astype(np.float16),
            (128, 1),
        )
    )

    in_maps = []
    for c in range(N_CORES):
        s = src[c * E_C : (c + 1) * E_C].astype(np.int32)
        d = dst[c * E_C : (c + 1) * E_C].astype(np.int32)
        pad = E_PAD - E_C
        if pad:
            s = np.concatenate([s, np.zeros(pad, np.int32)])
            d = np.concatenate([d, np.zeros(pad, np.int32)])
        # [128, N_TILES]: element [p, t] = index of edge t*128 + p
        s2 = np.ascontiguousarray(s.reshape(N_TILES, 128).T)
        d2 = np.ascontiguousarray(d.reshape(N_TILES, 128).T)
        in_maps.append(
            {
                "ht": ht,
                "wcat": wcat,
                "bcat": bcat,
                "src_idx": s2,
                "dst_idx": d2,
            }
        )
    return in_maps


def kernel(h, src, dst, W, b):
    if "nc" not in _CACHE:
        t0 = time.time()
        _CACHE["nc"] = _build_program()
        if os.environ.get("KERNEL_VERBOSE"):
            print(f"[kernel] build+compile: {time.time() - t0:.1f}s")
    nc = _CACHE["nc"]
    in_maps = _prep_inputs(h, src, dst, W, b)
    res = run_bass_kernel_spmd(nc, in_maps, core_ids=list(range(N_CORES)))
    outs = [res.results[c]["out"][:E_C] for c in range(N_CORES)]
    return np.concatenate(outs, axis=0)
